# revision 1
# baseline (speedup 1.0000x reference)
"""Trainium2 Bass kernel for Gemma4 MoE text-model backend (masks + gate).

Computes, for B=2, S=4096, H=2048, E=64, K=4:
  - full attention additive mask  [B,1,S,S]  (same-doc & causal)
  - sliding additive mask         [B,1,S,S]  ((causal window | same vision
    group) & same-doc)
  - MoE gate top-4 weights        [B*S, 4]
  - MoE gate top-4 indices        [B*S, 4]   int32

Sharding: pure data-parallel over the B*S=8192 query rows / tokens; core c
owns rows [1024c, 1024(c+1)).  Masks are generated on-device from small
per-position integer codes (doc/pos packed as qid*8192+pos, vision group as
qid*4096+gid) with tensor_scalar compares + ACT |.| range tests; the gate
GEMM runs on the PE via on-chip transposes, top-k via DVE max/max_index.
"""

import numpy as np

B = 2
S = 4096
H = 2048
E = 64
TOPK = 4
EPS = 1e-6
SW = 1024
NCORES = 8
ROWS = (B * S) // NCORES          # 1024 rows (= tokens) per core
P = 128                           # partitions
NTILES = ROWS // P                # 8 q-tiles per core
KCHUNK = 1024                     # kv chunk width for mask tiles
NCHUNK = S // KCHUNK              # 4 chunks per q-tile row
HCH = H // P                      # 16 contraction chunks for the gate GEMM

NEG = float(np.finfo(np.float32).min)

_NC_CACHE = {}


def _build_nc():
    import concourse.bacc as bacc
    import concourse.bass as bass
    import concourse.mybir as mybir
    import concourse.tile as tile
    from concourse.masks import make_identity

    A = mybir.AluOpType
    F = mybir.ActivationFunctionType
    f32 = mybir.dt.float32

    nc = bacc.Bacc("TRN2", target_bir_lowering=False)

    x_part = nc.dram_tensor("x_part", [ROWS, H], f32, kind="ExternalInput")
    tq_col_d = nc.dram_tensor("tq_col", [P, NTILES], f32, kind="ExternalInput")
    wq_col_d = nc.dram_tensor("wq_col", [P, NTILES], f32, kind="ExternalInput")
    tneg_row = nc.dram_tensor("tneg_row", [S], f32, kind="ExternalInput")
    w_row = nc.dram_tensor("w_row", [S], f32, kind="ExternalInput")
    wS = nc.dram_tensor("wS", [H, E], f32, kind="ExternalInput")

    full_part = nc.dram_tensor("full_part", [ROWS, S], f32, kind="ExternalOutput")
    slid_part = nc.dram_tensor("slid_part", [ROWS, S], f32, kind="ExternalOutput")
    w_out = nc.dram_tensor("w_out", [ROWS, TOPK], f32, kind="ExternalOutput")
    i_out = nc.dram_tensor("i_out", [ROWS, TOPK], mybir.dt.int32, kind="ExternalOutput")

    with tile.TileContext(nc) as tc:
        with (
            tc.tile_pool(name="singles", bufs=1) as singles,
            tc.tile_pool(name="mask", bufs=2) as mk,
            tc.tile_pool(name="gate", bufs=2) as gt,
            tc.tile_pool(name="gsmall", bufs=3) as gs,
            tc.tile_pool(name="psum_t", bufs=4, space="PSUM") as pst,
            tc.tile_pool(name="psum_l", bufs=2, space="PSUM") as psl,
        ):
            # ---------------- resident setup (all via gpsimd SWDGE) --------
            tneg_b = singles.tile([P, S], f32)
            nc.gpsimd.dma_start(
                out=tneg_b,
                in_=bass.AP(tensor=tneg_row, offset=0, ap=[[0, P], [1, S]]),
            )
            wrow_b = singles.tile([P, S], f32)
            nc.gpsimd.dma_start(
                out=wrow_b,
                in_=bass.AP(tensor=w_row, offset=0, ap=[[0, P], [1, S]]),
            )
            tq_col = singles.tile([P, NTILES], f32)
            nc.gpsimd.dma_start(out=tq_col, in_=tq_col_d[:, :])
            wq_col = singles.tile([P, NTILES], f32)
            nc.gpsimd.dma_start(out=wq_col, in_=wq_col_d[:, :])
            wS_sb = singles.tile([P, HCH, E], f32)
            nc.gpsimd.dma_start(out=wS_sb, in_=wS.rearrange("(c p) e -> p c e", p=P))

            ident = singles.tile([P, P], f32)
            make_identity(nc, ident)

            bias_full = singles.tile([P, 1], f32)
            nc.vector.memset(bias_full, -(S / 2 - 0.5))        # -2047.5
            bias_win = singles.tile([P, 1], f32)
            nc.vector.memset(bias_win, -(SW / 2 - 0.5))        # -511.5
            bias_eps = singles.tile([P, 1], f32)
            nc.vector.memset(bias_eps, EPS)

            # ---------------- masks ---------------------------------------
            # codes: t_q = qid*8192 + q ; tneg = -(kid*8192 + kv)
            #   full allowed  <=> t_q - t_kv in [0, S)    <=> |U - 2047.5| < 2048
            #   window allowed<=> t_q - t_kv in [0, SW)   <=> |U - 511.5| < 512
            #   vision allowed<=> w_q == w_kv
            for n in range(NTILES):
                for c in range(NCHUNK):
                    cs = slice(c * KCHUNK, (c + 1) * KCHUNK)
                    U = mk.tile([P, KCHUNK], f32, tag="U")
                    nc.vector.tensor_scalar(
                        out=U, in0=tneg_b[:, cs], scalar1=tq_col[:, n:n + 1],
                        scalar2=None, op0=A.add)
                    AF = mk.tile([P, KCHUNK], f32, tag="AF")
                    nc.scalar.activation(out=AF, in_=U, func=F.Abs,
                                         bias=bias_full[:, 0:1], scale=1.0)
                    FULL = mk.tile([P, KCHUNK], f32, tag="FULL")
                    nc.vector.tensor_scalar(
                        out=FULL, in0=AF, scalar1=float(S / 2), scalar2=NEG,
                        op0=A.is_ge, op1=A.mult)
                    nc.sync.dma_start(
                        out=full_part[n * P:(n + 1) * P, cs], in_=FULL)

                    AW = mk.tile([P, KCHUNK], f32, tag="AW")
                    nc.scalar.activation(out=AW, in_=U, func=F.Abs,
                                         bias=bias_win[:, 0:1], scale=1.0)
                    win01 = mk.tile([P, KCHUNK], f32, tag="win01")
                    nc.vector.tensor_scalar(
                        out=win01, in0=AW, scalar1=float(SW / 2), scalar2=None,
                        op0=A.is_ge)
                    vis01 = mk.tile([P, KCHUNK], f32, tag="vis01")
                    nc.vector.tensor_scalar(
                        out=vis01, in0=wrow_b[:, cs], scalar1=wq_col[:, n:n + 1],
                        scalar2=None, op0=A.not_equal)
                    MB = mk.tile([P, KCHUNK], f32, tag="MB")
                    nc.gpsimd.tensor_tensor(out=MB, in0=win01, in1=vis01,
                                            op=A.mult)
                    SLID = mk.tile([P, KCHUNK], f32, tag="SLID")
                    nc.vector.tensor_scalar(
                        out=SLID, in0=MB, scalar1=NEG, scalar2=None, op0=A.mult)
                    nc.sync.dma_start(
                        out=slid_part[n * P:(n + 1) * P, cs], in_=SLID)

            # ---------------- gate ----------------------------------------
            for n in range(NTILES):
                rs = slice(n * P, (n + 1) * P)
                x_t = gt.tile([P, H], f32, tag="x_t")
                nc.sync.dma_start(out=x_t, in_=x_part[rs, :])

                xsq = gt.tile([P, H], f32, tag="xsq")
                ssq = gs.tile([P, 1], f32, tag="ssq")
                nc.scalar.activation(out=xsq, in_=x_t, func=F.Square,
                                     accum_out=ssq)
                # rn = 1 / sqrt(ssq/H + eps)
                rms = gs.tile([P, 1], f32, tag="rms")
                nc.scalar.activation(out=rms, in_=ssq, func=F.Sqrt,
                                     bias=bias_eps[:, 0:1], scale=float(1.0 / H))
                rn = gs.tile([P, 1], f32, tag="rn")
                nc.vector.reciprocal(out=rn, in_=rms)

                logits_ps = psl.tile([P, E], f32, tag="logits_ps")
                for k in range(HCH):
                    xT_ps = pst.tile([P, P], f32, tag="xT_ps")
                    nc.tensor.transpose(xT_ps, x_t[:, k * P:(k + 1) * P], ident)
                    xT_sb = gt.tile([P, P], f32, tag="xT_sb")
                    nc.vector.tensor_copy(out=xT_sb, in_=xT_ps)
                    nc.tensor.matmul(logits_ps, lhsT=xT_sb, rhs=wS_sb[:, k, :],
                                     start=(k == 0), stop=(k == HCH - 1))

                logit = gs.tile([P, E], f32, tag="logit")
                nc.scalar.mul(logit, logits_ps, rn[:, 0:1])

                m8 = gs.tile([P, 8], f32, tag="m8")
                nc.vector.max(out=m8, in_=logit)
                i8 = gs.tile([P, 8], mybir.dt.uint32, tag="i8")
                nc.vector.max_index(out=i8, in_max=m8, in_values=logit)

                negm0 = gs.tile([P, 1], f32, tag="negm0")
                nc.scalar.mul(negm0, m8[:, 0:1], -1.0)
                e4 = gs.tile([P, TOPK], f32, tag="e4")
                nc.scalar.activation(out=e4, in_=m8[:, 0:TOPK], func=F.Exp,
                                     bias=negm0[:, 0:1], scale=1.0)
                s4 = gs.tile([P, 1], f32, tag="s4")
                nc.vector.reduce_sum(out=s4, in_=e4, axis=mybir.AxisListType.X)
                r4 = gs.tile([P, 1], f32, tag="r4")
                nc.vector.reciprocal(out=r4, in_=s4)
                w4 = gs.tile([P, TOPK], f32, tag="w4")
                nc.vector.tensor_scalar(out=w4, in0=e4, scalar1=r4[:, 0:1],
                                        scalar2=None, op0=A.mult)

                nc.sync.dma_start(out=w_out[rs, :], in_=w4)
                nc.sync.dma_start(out=i_out[rs, :],
                                  in_=i8[:, 0:TOPK].bitcast(mybir.dt.int32))

    nc.compile()
    return nc


def _get_nc():
    if "nc" not in _NC_CACHE:
        _NC_CACHE["nc"] = _build_nc()
    return _NC_CACHE["nc"]


def _host_precompute(packed, mm, scale, proj_w):
    """Small [B,S] integer code vectors + folded gate weights (all f32-exact)."""
    packed = np.asarray(packed).astype(np.int64)
    mm = np.asarray(mm).astype(np.int64)
    pos = np.arange(S, dtype=np.int64)[None, :]

    qid = np.where(packed > 0, packed, -1)
    kid = np.where(packed > 0, packed, -2)
    tq = (qid * (2 * S) + pos).astype(np.float32)
    tneg = (-(kid * (2 * S) + pos)).astype(np.float32)

    is_v = (mm == 1) | (mm == 2)
    prev = np.concatenate([np.zeros_like(is_v[:, :1]), is_v[:, :-1]], axis=1)
    starts = is_v & ~prev
    gid = np.cumsum(starts.astype(np.int64), axis=1) - 1
    validv = is_v & (packed > 0)
    wq = np.where(validv, qid * S + gid, -(2 * pos + 3)).astype(np.float32)
    wkv = np.where(validv, kid * S + gid, -(2 * pos + 4)).astype(np.float32)

    scale32 = np.asarray(scale, dtype=np.float32) * np.float32(H ** -0.5)
    wS = (np.asarray(proj_w, dtype=np.float32).T * scale32[:, None]).astype(np.float32)
    return tq, tneg, wq, wkv, wS


def kernel(x, packed_seq_ids, mm_token_type_ids, scale, proj_w):
    from concourse.bass_utils import run_bass_kernel_spmd

    x = np.ascontiguousarray(np.asarray(x, dtype=np.float32))
    assert x.shape == (B, S, H), x.shape
    tq, tneg, wq, wkv, wS = _host_precompute(
        packed_seq_ids, mm_token_type_ids, scale, proj_w)

    in_maps = []
    for c in range(NCORES):
        b = c // (NCORES // B)
        r0 = (c % (NCORES // B)) * ROWS
        rs = slice(r0, r0 + ROWS)
        in_maps.append({
            "x_part": np.ascontiguousarray(x[b, rs]),
            "tq_col": np.ascontiguousarray(tq[b, rs].reshape(NTILES, P).T),
            "wq_col": np.ascontiguousarray(wq[b, rs].reshape(NTILES, P).T),
            "tneg_row": np.ascontiguousarray(tneg[b]),
            "w_row": np.ascontiguousarray(wkv[b]),
            "wS": wS,
        })

    nc = _get_nc()
    res = run_bass_kernel_spmd(nc, in_maps, core_ids=list(range(NCORES)))

    full = np.empty((B, 1, S, S), dtype=np.float32)
    slid = np.empty((B, 1, S, S), dtype=np.float32)
    weights = np.empty((B * S, TOPK), dtype=np.float32)
    indices = np.empty((B * S, TOPK), dtype=np.int32)
    for c in range(NCORES):
        b = c // (NCORES // B)
        r0 = (c % (NCORES // B)) * ROWS
        out = res.results[c]
        full[b, 0, r0:r0 + ROWS] = out["full_part"]
        slid[b, 0, r0:r0 + ROWS] = out["slid_part"]
        weights[b * S + r0: b * S + r0 + ROWS] = out["w_out"]
        indices[b * S + r0: b * S + r0 + ROWS] = out["i_out"]
    return full, slid, weights, indices
